# revision 22
# baseline (speedup 1.0000x reference)
"""Multi-head attention (B=2, S=2048, D=1024, H=16, E=64) on 8 NeuronCores.

Sharding: core c = (batch b, head-group hg) with b = c // 4, hg = c % 4.
Each core projects q/k/v for its batch into its 4 heads, runs dense
attention for those heads over the full sequence, and computes a partial
output projection with its 256 rows of Wo.  The host sums the 4 partials
per batch and adds bo (the TP all-reduce, folded into the gather step).

v5 layout notes (everything "T" = feature-on-partitions):
  qhT/khT  [128, 2048] x2   BF16: head h at rows (h%2)*64 of j-block h//2.
  vh'      [128, 16*260]    BF16 per 128-key block: per head a 65-wide
                            block [vh | ones-col]; the ones column makes
                            the PV matmul emit the softmax denominator.
  scoresT  psum [t=128, 1024]  (512 q cols per head of the pair).
  exp      alternates between the ACT engine (true exp, bf16 out) and the
           Vector engine (Schraudolph bit-trick: i16 = round(s*A + B)
           reinterpreted as bf16 ~ exp(s/8), ~3% ripple that cancels in
           softmax) so the two engines split the 2048^2-per-head
           elementwise wall.
  phase order: k-proj, v-proj stream from HBM; xq parks in SBUF whole and
           q-projection chunks are interleaved into the attention stream
           (one 512-query chunk projected one chunk ahead).
  attn     8 BF16 block tiles [128, 512] (head-pair j x query-chunk) so
           normalization and the transposed out-projection pipeline at
           block granularity; out partials written [D, S] fp16, host sums.
"""

import numpy as np

B, S, D, H, E = 2, 2048, 1024, 16, 64
HG = 4            # heads per core
N_CORES = 8
EL = E + 1        # 65: head block width in vh' (values + ones column)
DT = D // 128     # 8 contraction tiles
SC = S // 512     # 4 s-chunks of 512

# Schraudolph exp on DVE: i16 = round(raw_score * ADVE + BDVE), bits are
# bf16(exp(raw_score/8)).  ADVE folds the 1/sqrt(E) score scale.
ADVE = 23.083120654223414      # (128/ln2) * 0.125
BDVE = 16250.5                 # 127*128 - C, C ~ 5.5 centers the ripple
N_DVE = 8                      # of 16 key-blocks per chunk exp'd on DVE

_NC = None        # cached compiled Bass module

# E_pair: 8 blocks [16, 128]; block (j, sc) broadcasts recip row (2j+m//64)*4+sc
# to output partition m — builds the per-head recip tile for a head-pair column
_EALL = np.zeros((16, 16 * E), np.float32)
for _j in range(2):
    for _sc in range(4):
        for _m in range(128):
            _EALL[(2 * _j + _m // 64) * 4 + _sc, (_j * 4 + _sc) * 128 + _m] = 1.0
_ONES = np.ones((1, 512), np.float32)


def _build():
    import concourse.bass as bass
    import concourse.mybir as mybir
    import concourse.tile as tile
    from concourse import bacc

    FP = mybir.dt.float32
    FPR = mybir.dt.float32r
    BF = mybir.dt.bfloat16
    F16 = mybir.dt.float16
    I16 = mybir.dt.int16
    EXP = mybir.ActivationFunctionType.Exp
    MULT = mybir.AluOpType.mult
    ADD = mybir.AluOpType.add

    nc = bacc.Bacc("TRN2", target_bir_lowering=False, debug=False, num_devices=1)

    xq = nc.dram_tensor("xq", [D, S], BF, kind="ExternalInput").ap()
    xk = nc.dram_tensor("xk", [D, S], BF, kind="ExternalInput").ap()
    xv = nc.dram_tensor("xv", [D, S], BF, kind="ExternalInput").ap()
    wq = nc.dram_tensor("wq", [D + 1, HG * E], BF, kind="ExternalInput").ap()
    wk = nc.dram_tensor("wk", [D + 1, HG * E], BF, kind="ExternalInput").ap()
    wv = nc.dram_tensor("wv", [D + 1, HG * EL], BF, kind="ExternalInput").ap()
    wo = nc.dram_tensor("wo", [HG * E, D], BF, kind="ExternalInput").ap()
    eall_d = nc.dram_tensor("eall", [16, 16 * E], FPR, kind="ExternalInput").ap()
    ones_d = nc.dram_tensor("ones", [1, 512], BF, kind="ExternalInput").ap()
    # out partial is [D, S] (features on rows): the transposed output
    # projection keeps wo stationary across 4 query-chunk matmuls.
    out = nc.dram_tensor("out_partial", [D, S], F16, kind="ExternalOutput").ap()

    with tile.TileContext(nc) as tc:
        with (
            tc.tile_pool(name="consts", bufs=1) as cpool,
            tc.tile_pool(name="resident", bufs=1) as rpool,
            tc.tile_pool(name="xin", bufs=6) as xpool,
            tc.tile_pool(name="xvin", bufs=8) as xvpool,
            tc.tile_pool(name="expa", bufs=3) as epool_a,
            tc.tile_pool(name="expv", bufs=3) as epool_v,
            tc.tile_pool(name="stage", bufs=4) as spool,
            tc.tile_pool(name="outev", bufs=4) as opool,
        ):
            ones = cpool.tile([1, 512], BF, tag="ones")
            nc.gpsimd.dma_start(ones[:], ones_d[:])

            wk_sb = cpool.tile([128, DT * 256], BF, tag="wk")
            wv_sb = cpool.tile([128, DT * 260], BF, tag="wv")
            wq_sb = cpool.tile([128, DT * 256], BF, tag="wq")
            wqb = cpool.tile([1, 256], BF, tag="wqb")
            wkb = cpool.tile([1, 256], BF, tag="wkb")
            wvb = cpool.tile([1, 260], BF, tag="wvb")
            # wk on the scalar queue ahead of xv so k-proj starts earliest;
            # wv/wq and the rest on gpsimd; xk owns the sync queue.
            for dt in range(DT):
                nc.scalar.dma_start(
                    wk_sb[:, dt * 256 : (dt + 1) * 256],
                    wk[dt * 128 : (dt + 1) * 128, :],
                )
            nc.scalar.dma_start(wkb[:], wk[D : D + 1, :])
            for dt in range(DT):
                nc.gpsimd.dma_start(
                    wv_sb[:, dt * 260 : (dt + 1) * 260],
                    wv[dt * 128 : (dt + 1) * 128, :],
                )
            nc.gpsimd.dma_start(wvb[:], wv[D : D + 1, :])
            for dt in range(DT):
                nc.gpsimd.dma_start(
                    wq_sb[:, dt * 256 : (dt + 1) * 256],
                    wq[dt * 128 : (dt + 1) * 128, :],
                )
            nc.gpsimd.dma_start(wqb[:], wq[D : D + 1, :])

            # E_all[k, r*64+j] = (k == r): broadcasts recip row r via matmul
            e_all = cpool.tile([16, 16 * E], FPR, tag="eall")
            nc.gpsimd.dma_start(e_all[:], eall_d[:])

            wo_sb = []
            for j in range(2):
                t = cpool.tile([128, D], BF, tag=f"wo{j}")
                nc.gpsimd.dma_start(t[:], wo[j * 128 : (j + 1) * 128, :])
                wo_sb.append(t)

            # xq parks fully in SBUF (32KB/partition); q-proj chunks read it
            # with no per-chunk DMA dependency.
            xq_sb = rpool.tile([128, DT * 2048], BF, tag="xq_sb")
            for dt in range(DT):
                nc.gpsimd.dma_start(
                    xq_sb[:, dt * 2048 : (dt + 1) * 2048],
                    xq[dt * 128 : (dt + 1) * 128, :],
                )

            qhT = rpool.tile([128, 2 * S], BF, tag="qhT")
            khT = rpool.tile([128, 2 * S], BF, tag="khT")
            vh = rpool.tile([128, 16 * 260], BF, tag="vh")
            attn = {}
            for j in range(2):
                for sc in range(SC):
                    attn[j, sc] = rpool.tile(
                        [128, 512], BF, tag=f"attn_{j}_{sc}", name=f"attn_{j}_{sc}"
                    )
            sums = rpool.tile([16, 512], BF, tag="sums")
            recip = rpool.tile([16, 512], FPR, tag="recip")

            # ---- phase A: k and v projections ----------------------------
            # k: dt-outer with all 8 (j, sc) psums open, so one weight
            # load (lhsT) serves 4 s-chunk matmuls.
            with tc.tile_pool(name="ps_proj", bufs=8, space="PSUM") as pp:
                pss = {}
                for j in range(2):
                    for sc in range(SC):
                        pss[j, sc] = pp.tile(
                            [128, 512], FP, tag="pp", name=f"pp_{j}_{sc}"
                        )
                for dt in range(DT):
                    t = xpool.tile([128, 2048], BF, tag="xin")
                    keng = nc.sync if dt % 2 == 0 else nc.scalar
                    keng.dma_start(t[:], xk[dt * 128 : (dt + 1) * 128, :])
                    for j in range(2):
                        for sc in range(SC):
                            nc.tensor.matmul(
                                pss[j, sc][:],
                                wk_sb[:, dt * 256 + j * 128 : dt * 256 + (j + 1) * 128],
                                t[:, sc * 512 : (sc + 1) * 512],
                                start=(dt == 0),
                                stop=False,
                            )
                for j in range(2):
                    for sc in range(SC):
                        nc.tensor.matmul(
                            pss[j, sc][:],
                            wkb[0:1, j * 128 : (j + 1) * 128],
                            ones[0:1, :],
                            start=False,
                            stop=True,
                        )
                        dslice = khT[:, j * S + sc * 512 : j * S + (sc + 1) * 512]
                        if (j * SC + sc) % 2 == 0:
                            nc.vector.tensor_copy(dslice, pss[j, sc][:])
                        else:
                            nc.scalar.copy(dslice, pss[j, sc][:])
            # v: vh' tiles [t=128, 260] per 128-key block; dt-inner with
            # the 8 key-block psums of one 1024-key half open.
            with tc.tile_pool(name="ps_vproj", bufs=8, space="PSUM") as pv:
                for half in range(2):
                    psv = [
                        pv.tile([128, 260], FP, tag="ppv", name=f"ppv_{half}_{u}")
                        for u in range(8)
                    ]
                    for dt in range(DT):
                        t = xvpool.tile([128, 1024], BF, tag="xvin")
                        eng = nc.sync if dt % 2 == 0 else nc.scalar
                        eng.dma_start(
                            t[:],
                            xv[
                                dt * 128 : (dt + 1) * 128,
                                half * 1024 : (half + 1) * 1024,
                            ],
                        )
                        for u in range(8):
                            nc.tensor.matmul(
                                psv[u][:],
                                t[:, u * 128 : (u + 1) * 128],
                                wv_sb[:, dt * 260 : (dt + 1) * 260],
                                start=(dt == 0),
                                stop=False,
                            )
                    for u in range(8):
                        tt = half * 8 + u
                        nc.tensor.matmul(
                            psv[u][:],
                            ones[0:1, 0:128],
                            wvb[0:1, :],
                            start=False,
                            stop=True,
                        )
                        vslice = vh[:, tt * 260 : (tt + 1) * 260]
                        if u % 2 == 0:
                            nc.vector.tensor_copy(vslice, psv[u][:])
                        else:
                            nc.scalar.copy(vslice, psv[u][:])

            # ---- phase C: q-proj chunks interleaved with attention -------
            with (
                tc.tile_pool(name="ps_sc", bufs=3, space="PSUM") as psc,
                tc.tile_pool(name="ps_pv", bufs=2, space="PSUM") as ppv,
            ):

                def qproj(j, sc):
                    # borrows a scores-pool psum tile (uses its first bank)
                    qp = psc.tile([128, 1024], FP, tag="sc", name=f"qp_{j}_{sc}")
                    ps = qp[:, 0:512]
                    for dt in range(DT):
                        nc.tensor.matmul(
                            ps,
                            wq_sb[:, dt * 256 + j * 128 : dt * 256 + (j + 1) * 128],
                            xq_sb[:, dt * 2048 + sc * 512 : dt * 2048 + (sc + 1) * 512],
                            start=(dt == 0),
                            stop=False,
                        )
                    nc.tensor.matmul(
                        ps,
                        wqb[0:1, j * 128 : (j + 1) * 128],
                        ones[0:1, :],
                        start=False,
                        stop=True,
                    )
                    dslice = qhT[:, j * S + sc * 512 : j * S + (sc + 1) * 512]
                    if (j * SC + sc) % 2 == 0:
                        nc.vector.tensor_copy(dslice, ps)
                    else:
                        nc.scalar.copy(dslice, ps)

                def attention(hp, sc):
                    h0, h1 = 2 * hp, 2 * hp + 1
                    pv0 = ppv.tile([EL, 512], FP, tag="pv", name=f"pv0_{hp}_{sc}")
                    pv1 = ppv.tile([EL, 512], FP, tag="pv", name=f"pv1_{hp}_{sc}")
                    exq = []

                    def scores(tt):
                        ps = psc.tile([128, 1024], FP, tag="sc")
                        nc.tensor.matmul(
                            ps[:, 0:512],
                            khT[0:64, hp * S + tt * 128 : hp * S + (tt + 1) * 128],
                            qhT[0:64, hp * S + sc * 512 : hp * S + (sc + 1) * 512],
                            start=True,
                            stop=True,
                        )
                        nc.tensor.matmul(
                            ps[:, 512:1024],
                            khT[64:128, hp * S + tt * 128 : hp * S + (tt + 1) * 128],
                            qhT[64:128, hp * S + sc * 512 : hp * S + (sc + 1) * 512],
                            start=True,
                            stop=True,
                        )
                        # alternate exp between DVE (Schraudolph bit trick)
                        # and ACT so both engines split the wall
                        if tt % 2 == 0 and tt < 2 * N_DVE:
                            ex = epool_v.tile([128, 1024], I16, tag="expv")
                            nc.vector.tensor_scalar(
                                ex[:], ps[:], ADVE, BDVE, MULT, ADD
                            )
                            exq.append((ex, True))
                        else:
                            ex = epool_a.tile([128, 1024], BF, tag="expa")
                            nc.scalar.activation(ex[:], ps[:], EXP, scale=0.125)
                            exq.append((ex, False))

                    def pv_ex(tt, head):
                        ex, is_i16 = exq[tt]
                        sl = ex[:, head * 512 : (head + 1) * 512]
                        return sl.bitcast(BF) if is_i16 else sl

                    def pv_a(tt):
                        nc.tensor.matmul(
                            pv0[:],
                            vh[:, tt * 260 + (h0 % 4) * EL : tt * 260 + (h0 % 4) * EL + EL],
                            pv_ex(tt, 0),
                            start=(tt == 0),
                            stop=(tt == 15),
                        )

                    def pv_b(tt):
                        nc.tensor.matmul(
                            pv1[:],
                            vh[:, tt * 260 + (h1 % 4) * EL : tt * 260 + (h1 % 4) * EL + EL],
                            pv_ex(tt, 1),
                            start=(tt == 0),
                            stop=(tt == 15),
                        )

                    # scores run two key-blocks ahead of PV so exp(tt) and
                    # exp(tt+1) overlap on their two engines; the score pair
                    # is emitted between pv0 and pv1 so every LDWEIGHTS can
                    # load under the previous matmul's stream.
                    scores(0)
                    scores(1)
                    for tt in range(14):
                        pv_a(tt)
                        scores(tt + 2)
                        pv_b(tt)
                    pv_a(14)
                    pv_b(14)
                    pv_a(15)
                    pv_b(15)

                    r0, r1 = h0 * SC + sc, h1 * SC + sc
                    st0 = spool.tile([EL, 512], BF, tag="stage")
                    st1 = spool.tile([EL, 512], BF, tag="stage")
                    nc.vector.tensor_copy(st0[:], pv0[:])
                    nc.scalar.copy(st1[:], pv1[:])
                    nc.gpsimd.dma_start(attn[hp, sc][0:64, :], st0[0:E, :])
                    nc.gpsimd.dma_start(attn[hp, sc][64:128, :], st1[0:E, :])
                    nc.gpsimd.dma_start(sums[r0 : r0 + 1, :], st0[E : E + 1, :])
                    nc.gpsimd.dma_start(sums[r1 : r1 + 1, :], st1[E : E + 1, :])

                chunks = [(hp, sc) for hp in range(2) for sc in range(SC)]
                qproj(*chunks[0])
                for ci, (hp, sc) in enumerate(chunks):
                    if ci + 1 < len(chunks):
                        qproj(*chunks[ci + 1])
                    attention(hp, sc)

            # ---- phase D: normalize + output projection ------------------
            with nc.allow_low_precision(reason="bf16 sums -> fp32r recip"):
                nc.vector.reciprocal(recip[:], sums[:])
            with (
                tc.tile_pool(name="ps_rb", bufs=2, space="PSUM") as prb,
                tc.tile_pool(name="ps_op", bufs=6, space="PSUM") as pop,
            ):
                # normalization qc-major so early query-chunks unblock the
                # out-projection first
                for sc in range(SC):
                    for j in range(2):
                        rb = prb.tile([128, 512], FP, tag="rb")
                        nc.tensor.matmul(
                            rb[:],
                            e_all[:, (j * 4 + sc) * 128 : (j * 4 + sc + 1) * 128],
                            recip[:],
                            start=True,
                            stop=True,
                        )
                        a = attn[j, sc]
                        nc.vector.tensor_mul(a[:], a[:], rb[:])
                # transposed out-projection: query-pair outer so the first
                # normalized chunks stream out while later ones normalize.
                for qp in range(2):
                    for db in range(8):
                        pA = pop.tile([128, 512], FP, tag="op", name=f"opA_{qp}_{db}")
                        pB = pop.tile([128, 512], FP, tag="op", name=f"opB_{qp}_{db}")
                        for j in range(2):
                            nc.tensor.matmul(
                                pA[:],
                                wo_sb[j][:, db * 128 : (db + 1) * 128],
                                attn[j, 2 * qp][:],
                                start=(j == 0),
                                stop=(j == 1),
                            )
                            nc.tensor.matmul(
                                pB[:],
                                wo_sb[j][:, db * 128 : (db + 1) * 128],
                                attn[j, 2 * qp + 1][:],
                                start=(j == 0),
                                stop=(j == 1),
                            )
                        ot = opool.tile([128, 1024], F16, tag="outev")
                        if db % 2 == 0:
                            nc.vector.tensor_copy(ot[:, 0:512], pA[:])
                            nc.scalar.copy(ot[:, 512:1024], pB[:])
                        else:
                            nc.scalar.copy(ot[:, 0:512], pA[:])
                            nc.vector.tensor_copy(ot[:, 512:1024], pB[:])
                        deng = nc.sync if db % 2 == 0 else nc.gpsimd
                        deng.dma_start(
                            out[db * 128 : (db + 1) * 128, qp * 1024 : (qp + 1) * 1024],
                            ot[:],
                        )

    nc.compile()
    return nc


def _get_nc():
    global _NC
    if _NC is None:
        _NC = _build()
    return _NC


def _in_maps(q, k, v, Wq, bq, Wk, bk, Wv, bv, Wo, bo):
    import ml_dtypes
    f32 = np.float32
    bf16 = ml_dtypes.bfloat16
    maps = []
    for c in range(N_CORES):
        b, hg = c // HG, c % HG
        hs = slice(hg * HG, (hg + 1) * HG)  # this core's 4 heads

        wq_h = np.zeros((D + 1, HG * E), f32)
        wq_h[:D] = np.transpose(Wq[hs], (1, 0, 2)).reshape(D, HG * E)
        wq_h[D] = bq[hs].reshape(-1)
        wk_h = np.zeros((D + 1, HG * E), f32)
        wk_h[:D] = np.transpose(Wk[hs], (1, 0, 2)).reshape(D, HG * E)
        wk_h[D] = bk[hs].reshape(-1)
        wv_h = np.zeros((D + 1, HG * EL), f32)
        for hl in range(HG):
            wv_h[:D, hl * EL : hl * EL + E] = Wv[hg * HG + hl]
            wv_h[D, hl * EL : hl * EL + E] = bv[hg * HG + hl]
            wv_h[D, hl * EL + E] = 1.0  # generates the ones column of vh'
        maps.append(
            {
                "xq": np.ascontiguousarray(q[b].T).astype(bf16),
                "xk": np.ascontiguousarray(k[b].T).astype(bf16),
                "xv": np.ascontiguousarray(v[b].T).astype(bf16),
                "wq": wq_h.astype(bf16),
                "wk": wk_h.astype(bf16),
                "wv": wv_h.astype(bf16),
                "wo": np.ascontiguousarray(
                    Wo[hg * HG * E : (hg + 1) * HG * E, :]
                ).astype(bf16),
                "eall": _EALL,
                "ones": _ONES.astype(bf16),
            }
        )
    return maps


def _run(inputs, trace=False):
    from concourse.bass_utils import run_bass_kernel_spmd

    nc = _get_nc()
    maps = _in_maps(**inputs)
    res = run_bass_kernel_spmd(nc, maps, list(range(N_CORES)), trace=trace)
    bo = np.asarray(inputs["bo"], np.float32)
    out = np.zeros((B, S, D), np.float32)
    for b in range(B):
        acc = np.zeros((D, S), np.float32)
        for hg in range(HG):
            acc += res.results[b * HG + hg]["out_partial"].astype(np.float32)
        out[b] = acc.T + bo[None, :]
    return out, res.exec_time_ns


def kernel(**inputs):
    out, _ = _run(inputs, trace=False)
    return out


def kernel_traced(**inputs):
    return _run(inputs, trace=True)


# revision 23
# speedup vs baseline: 1.1280x; 1.1280x over previous
"""Multi-head attention (B=2, S=2048, D=1024, H=16, E=64) on 8 NeuronCores.

Sharding: core c = (batch b, head-group hg) with b = c // 4, hg = c % 4.
Each core projects q/k/v for its batch into its 4 heads, runs dense
attention for those heads over the full sequence, and computes a partial
output projection with its 256 rows of Wo.  The host sums the 4 partials
per batch and adds bo (the TP all-reduce, folded into the gather step).

v5 layout notes (everything "T" = feature-on-partitions):
  qhT/khT  [128, 2048] x2   BF16: head h at rows (h%2)*64 of j-block h//2.
  vh'      [128, 16*260]    BF16 per 128-key block: per head a 65-wide
                            block [vh | ones-col]; the ones column makes
                            the PV matmul emit the softmax denominator.
  scoresT  psum [t=128, 1024]  (512 q cols per head of the pair).
  exp      alternates between the ACT engine (true exp, bf16 out) and the
           Vector engine (Schraudolph bit-trick: i16 = round(s*A + B)
           reinterpreted as bf16 ~ exp(s/8), ~3% ripple that cancels in
           softmax) so the two engines split the 2048^2-per-head
           elementwise wall.
  phase order: k-proj, v-proj stream from HBM; xq parks in SBUF whole and
           q-projection chunks are interleaved into the attention stream
           (one 512-query chunk projected one chunk ahead).
  attn     8 BF16 block tiles [128, 512] (head-pair j x query-chunk) so
           normalization and the transposed out-projection pipeline at
           block granularity; out partials written [D, S] fp16, host sums.
"""

import numpy as np

B, S, D, H, E = 2, 2048, 1024, 16, 64
HG = 4            # heads per core
N_CORES = 8
EL = E + 1        # 65: head block width in vh' (values + ones column)
DT = D // 128     # 8 contraction tiles
SC = S // 512     # 4 s-chunks of 512

# Schraudolph exp on DVE: i16 = round(raw_score * ADVE + BDVE), bits are
# bf16(exp(raw_score/8)).  ADVE folds the 1/sqrt(E) score scale.
ADVE = 23.083120654223414      # (128/ln2) * 0.125
BDVE = 16250.5                 # 127*128 - C, C ~ 5.5 centers the ripple
N_DVE = 8                      # of 16 key-blocks per chunk exp'd on DVE

_NC = None        # cached compiled Bass module

# E_pair: 8 blocks [16, 128]; block (j, sc) broadcasts recip row (2j+m//64)*4+sc
# to output partition m — builds the per-head recip tile for a head-pair column
_EALL = np.zeros((16, 16 * E), np.float32)
for _j in range(2):
    for _sc in range(4):
        for _m in range(128):
            _EALL[(2 * _j + _m // 64) * 4 + _sc, (_j * 4 + _sc) * 128 + _m] = 1.0
_ONES = np.ones((1, 512), np.float32)


def _build():
    import concourse.bass as bass
    import concourse.mybir as mybir
    import concourse.tile as tile
    from concourse import bacc

    FP = mybir.dt.float32
    FPR = mybir.dt.float32r
    BF = mybir.dt.bfloat16
    F16 = mybir.dt.float16
    I16 = mybir.dt.int16
    EXP = mybir.ActivationFunctionType.Exp
    MULT = mybir.AluOpType.mult
    ADD = mybir.AluOpType.add

    nc = bacc.Bacc("TRN2", target_bir_lowering=False, debug=False, num_devices=1)

    xq = nc.dram_tensor("xq", [D, S], BF, kind="ExternalInput").ap()
    xk = nc.dram_tensor("xk", [D, S], BF, kind="ExternalInput").ap()
    xv = nc.dram_tensor("xv", [D, S], BF, kind="ExternalInput").ap()
    wq = nc.dram_tensor("wq", [D + 1, HG * E], BF, kind="ExternalInput").ap()
    wk = nc.dram_tensor("wk", [D + 1, HG * E], BF, kind="ExternalInput").ap()
    wv = nc.dram_tensor("wv", [D + 1, HG * EL], BF, kind="ExternalInput").ap()
    wo = nc.dram_tensor("wo", [HG * E, D], BF, kind="ExternalInput").ap()
    eall_d = nc.dram_tensor("eall", [16, 16 * E], FPR, kind="ExternalInput").ap()
    ones_d = nc.dram_tensor("ones", [1, 512], BF, kind="ExternalInput").ap()
    # out partial is [D, S] (features on rows): the transposed output
    # projection keeps wo stationary across 4 query-chunk matmuls.
    out = nc.dram_tensor("out_partial", [D, S], F16, kind="ExternalOutput").ap()

    with tile.TileContext(nc) as tc:
        with (
            tc.tile_pool(name="consts", bufs=1) as cpool,
            tc.tile_pool(name="resident", bufs=1) as rpool,
            tc.tile_pool(name="xin", bufs=6) as xpool,
            tc.tile_pool(name="xvin", bufs=8) as xvpool,
            tc.tile_pool(name="expa", bufs=3) as epool_a,
            tc.tile_pool(name="expv", bufs=3) as epool_v,
            tc.tile_pool(name="stage", bufs=4) as spool,
            tc.tile_pool(name="outev", bufs=4) as opool,
        ):
            ones = cpool.tile([1, 512], BF, tag="ones")
            nc.gpsimd.dma_start(ones[:], ones_d[:])

            wk_sb = cpool.tile([128, DT * 256], BF, tag="wk")
            wv_sb = cpool.tile([128, DT * 260], BF, tag="wv")
            wq_sb = cpool.tile([128, DT * 256], BF, tag="wq")
            wqb = cpool.tile([1, 256], BF, tag="wqb")
            wkb = cpool.tile([1, 256], BF, tag="wkb")
            wvb = cpool.tile([1, 260], BF, tag="wvb")
            # wk on the scalar queue ahead of xv so k-proj starts earliest;
            # wv/wq and the rest on gpsimd; xk owns the sync queue.
            for dt in range(DT):
                nc.scalar.dma_start(
                    wk_sb[:, dt * 256 : (dt + 1) * 256],
                    wk[dt * 128 : (dt + 1) * 128, :],
                )
            nc.scalar.dma_start(wkb[:], wk[D : D + 1, :])
            for dt in range(DT):
                nc.gpsimd.dma_start(
                    wv_sb[:, dt * 260 : (dt + 1) * 260],
                    wv[dt * 128 : (dt + 1) * 128, :],
                )
            nc.gpsimd.dma_start(wvb[:], wv[D : D + 1, :])
            for dt in range(DT):
                nc.gpsimd.dma_start(
                    wq_sb[:, dt * 256 : (dt + 1) * 256],
                    wq[dt * 128 : (dt + 1) * 128, :],
                )
            nc.gpsimd.dma_start(wqb[:], wq[D : D + 1, :])

            # E_all[k, r*64+j] = (k == r): broadcasts recip row r via matmul
            e_all = cpool.tile([16, 16 * E], FPR, tag="eall")
            nc.gpsimd.dma_start(e_all[:], eall_d[:])

            wo_sb = []
            for j in range(2):
                t = cpool.tile([128, D], BF, tag=f"wo{j}")
                nc.gpsimd.dma_start(t[:], wo[j * 128 : (j + 1) * 128, :])
                wo_sb.append(t)

            # xq parks fully in SBUF (32KB/partition); q-proj chunks read it
            # with no per-chunk DMA dependency.
            xq_sb = rpool.tile([128, DT * 2048], BF, tag="xq_sb")
            for dt in range(DT):
                nc.gpsimd.dma_start(
                    xq_sb[:, dt * 2048 : (dt + 1) * 2048],
                    xq[dt * 128 : (dt + 1) * 128, :],
                )

            qhT = rpool.tile([128, 2 * S], BF, tag="qhT")
            khT = rpool.tile([128, 2 * S], BF, tag="khT")
            vh = rpool.tile([128, 16 * 260], BF, tag="vh")
            attn = {}
            for j in range(2):
                for sc in range(SC):
                    attn[j, sc] = rpool.tile(
                        [128, 512], BF, tag=f"attn_{j}_{sc}", name=f"attn_{j}_{sc}"
                    )
            sums = rpool.tile([16, 512], BF, tag="sums")
            recip = rpool.tile([16, 512], FPR, tag="recip")

            # ---- phase A: k and v projections ----------------------------
            # k: dt-outer with all 8 (j, sc) psums open, so one weight
            # load (lhsT) serves 4 s-chunk matmuls.
            with tc.tile_pool(name="ps_proj", bufs=8, space="PSUM") as pp:
                pss = {}
                for j in range(2):
                    for sc in range(SC):
                        pss[j, sc] = pp.tile(
                            [128, 512], FP, tag="pp", name=f"pp_{j}_{sc}"
                        )
                for dt in range(DT):
                    t = xpool.tile([128, 2048], BF, tag="xin")
                    keng = nc.sync if dt % 2 == 0 else nc.scalar
                    keng.dma_start(t[:], xk[dt * 128 : (dt + 1) * 128, :])
                    for j in range(2):
                        for sc in range(SC):
                            nc.tensor.matmul(
                                pss[j, sc][:],
                                wk_sb[:, dt * 256 + j * 128 : dt * 256 + (j + 1) * 128],
                                t[:, sc * 512 : (sc + 1) * 512],
                                start=(dt == 0),
                                stop=False,
                            )
                for j in range(2):
                    for sc in range(SC):
                        nc.tensor.matmul(
                            pss[j, sc][:],
                            wkb[0:1, j * 128 : (j + 1) * 128],
                            ones[0:1, :],
                            start=False,
                            stop=True,
                        )
                        dslice = khT[:, j * S + sc * 512 : j * S + (sc + 1) * 512]
                        if (j * SC + sc) % 2 == 0:
                            nc.vector.tensor_copy(dslice, pss[j, sc][:])
                        else:
                            nc.scalar.copy(dslice, pss[j, sc][:])
            # v: vh' tiles [t=128, 260] per 128-key block; dt-inner with
            # the 8 key-block psums of one 1024-key half open.
            with tc.tile_pool(name="ps_vproj", bufs=8, space="PSUM") as pv:
                for half in range(2):
                    psv = [
                        pv.tile([128, 260], FP, tag="ppv", name=f"ppv_{half}_{u}")
                        for u in range(8)
                    ]
                    for dt in range(DT):
                        t = xvpool.tile([128, 1024], BF, tag="xvin")
                        eng = nc.sync if dt % 2 == 0 else nc.scalar
                        eng.dma_start(
                            t[:],
                            xv[
                                dt * 128 : (dt + 1) * 128,
                                half * 1024 : (half + 1) * 1024,
                            ],
                        )
                        for u in range(8):
                            nc.tensor.matmul(
                                psv[u][:],
                                t[:, u * 128 : (u + 1) * 128],
                                wv_sb[:, dt * 260 : (dt + 1) * 260],
                                start=(dt == 0),
                                stop=False,
                            )
                    for u in range(8):
                        tt = half * 8 + u
                        nc.tensor.matmul(
                            psv[u][:],
                            ones[0:1, 0:128],
                            wvb[0:1, :],
                            start=False,
                            stop=True,
                        )
                        vslice = vh[:, tt * 260 : (tt + 1) * 260]
                        if u % 2 == 0:
                            nc.vector.tensor_copy(vslice, psv[u][:])
                        else:
                            nc.scalar.copy(vslice, psv[u][:])

            # ---- phase C: q-proj chunks interleaved with attention -------
            with (
                tc.tile_pool(name="ps_sc", bufs=3, space="PSUM") as psc,
                tc.tile_pool(name="ps_pv", bufs=2, space="PSUM") as ppv,
            ):

                def qproj(j, sc):
                    # borrows a scores-pool psum tile (uses its first bank)
                    qp = psc.tile([128, 1024], FP, tag="sc", name=f"qp_{j}_{sc}")
                    ps = qp[:, 0:512]
                    for dt in range(DT):
                        nc.tensor.matmul(
                            ps,
                            wq_sb[:, dt * 256 + j * 128 : dt * 256 + (j + 1) * 128],
                            xq_sb[:, dt * 2048 + sc * 512 : dt * 2048 + (sc + 1) * 512],
                            start=(dt == 0),
                            stop=False,
                        )
                    nc.tensor.matmul(
                        ps,
                        wqb[0:1, j * 128 : (j + 1) * 128],
                        ones[0:1, :],
                        start=False,
                        stop=True,
                    )
                    dslice = qhT[:, j * S + sc * 512 : j * S + (sc + 1) * 512]
                    if (j * SC + sc) % 2 == 0:
                        nc.vector.tensor_copy(dslice, ps)
                    else:
                        nc.scalar.copy(dslice, ps)

                def attention(hp, sc):
                    h0, h1 = 2 * hp, 2 * hp + 1
                    pv0 = ppv.tile([EL, 512], FP, tag="pv", name=f"pv0_{hp}_{sc}")
                    pv1 = ppv.tile([EL, 512], FP, tag="pv", name=f"pv1_{hp}_{sc}")
                    exq = []

                    def scores(tt):
                        ps = psc.tile([128, 1024], FP, tag="sc")
                        nc.tensor.matmul(
                            ps[:, 0:512],
                            khT[0:64, hp * S + tt * 128 : hp * S + (tt + 1) * 128],
                            qhT[0:64, hp * S + sc * 512 : hp * S + (sc + 1) * 512],
                            start=True,
                            stop=True,
                        )
                        nc.tensor.matmul(
                            ps[:, 512:1024],
                            khT[64:128, hp * S + tt * 128 : hp * S + (tt + 1) * 128],
                            qhT[64:128, hp * S + sc * 512 : hp * S + (sc + 1) * 512],
                            start=True,
                            stop=True,
                        )
                        # alternate exp between DVE (Schraudolph bit trick)
                        # and ACT so both engines split the wall
                        if tt % 2 == 0 and tt < 2 * N_DVE:
                            ex = epool_v.tile([128, 1024], I16, tag="expv")
                            nc.vector.tensor_scalar(
                                ex[:], ps[:], ADVE, BDVE, MULT, ADD
                            )
                            exq.append((ex, True))
                        else:
                            ex = epool_a.tile([128, 1024], BF, tag="expa")
                            nc.scalar.activation(ex[:], ps[:], EXP, scale=0.125)
                            exq.append((ex, False))

                    def pv_ex(tt, head):
                        ex, is_i16 = exq[tt]
                        sl = ex[:, head * 512 : (head + 1) * 512]
                        return sl.bitcast(BF) if is_i16 else sl

                    def pv_a(tt):
                        nc.tensor.matmul(
                            pv0[:],
                            vh[:, tt * 260 + (h0 % 4) * EL : tt * 260 + (h0 % 4) * EL + EL],
                            pv_ex(tt, 0),
                            start=(tt == 0),
                            stop=(tt == 15),
                        )

                    def pv_b(tt):
                        nc.tensor.matmul(
                            pv1[:],
                            vh[:, tt * 260 + (h1 % 4) * EL : tt * 260 + (h1 % 4) * EL + EL],
                            pv_ex(tt, 1),
                            start=(tt == 0),
                            stop=(tt == 15),
                        )

                    # scores run two key-blocks ahead of PV so exp(tt) and
                    # exp(tt+1) overlap on their two engines while PV(tt-1)
                    # waits only on the older exp.
                    scores(0)
                    scores(1)
                    for tt in range(14):
                        pv_a(tt)
                        pv_b(tt)
                        scores(tt + 2)
                    pv_a(14)
                    pv_b(14)
                    pv_a(15)
                    pv_b(15)

                    r0, r1 = h0 * SC + sc, h1 * SC + sc
                    st0 = spool.tile([EL, 512], BF, tag="stage")
                    st1 = spool.tile([EL, 512], BF, tag="stage")
                    nc.vector.tensor_copy(st0[:], pv0[:])
                    nc.scalar.copy(st1[:], pv1[:])
                    nc.gpsimd.dma_start(attn[hp, sc][0:64, :], st0[0:E, :])
                    nc.gpsimd.dma_start(attn[hp, sc][64:128, :], st1[0:E, :])
                    nc.gpsimd.dma_start(sums[r0 : r0 + 1, :], st0[E : E + 1, :])
                    nc.gpsimd.dma_start(sums[r1 : r1 + 1, :], st1[E : E + 1, :])

                chunks = [(hp, sc) for hp in range(2) for sc in range(SC)]
                qproj(*chunks[0])
                for ci, (hp, sc) in enumerate(chunks):
                    if ci + 1 < len(chunks):
                        qproj(*chunks[ci + 1])
                    attention(hp, sc)

            # ---- phase D: normalize + output projection ------------------
            with nc.allow_low_precision(reason="bf16 sums -> fp32r recip"):
                nc.vector.reciprocal(recip[:], sums[:])
            with (
                tc.tile_pool(name="ps_rb", bufs=2, space="PSUM") as prb,
                tc.tile_pool(name="ps_op", bufs=6, space="PSUM") as pop,
            ):
                # normalization qc-major so early query-chunks unblock the
                # out-projection first
                for sc in range(SC):
                    for j in range(2):
                        rb = prb.tile([128, 512], FP, tag="rb")
                        nc.tensor.matmul(
                            rb[:],
                            e_all[:, (j * 4 + sc) * 128 : (j * 4 + sc + 1) * 128],
                            recip[:],
                            start=True,
                            stop=True,
                        )
                        a = attn[j, sc]
                        nc.vector.tensor_mul(a[:], a[:], rb[:])
                # transposed out-projection: query-pair outer so the first
                # normalized chunks stream out while later ones normalize.
                for qp in range(2):
                    for db in range(8):
                        pA = pop.tile([128, 512], FP, tag="op", name=f"opA_{qp}_{db}")
                        pB = pop.tile([128, 512], FP, tag="op", name=f"opB_{qp}_{db}")
                        for j in range(2):
                            nc.tensor.matmul(
                                pA[:],
                                wo_sb[j][:, db * 128 : (db + 1) * 128],
                                attn[j, 2 * qp][:],
                                start=(j == 0),
                                stop=(j == 1),
                            )
                            nc.tensor.matmul(
                                pB[:],
                                wo_sb[j][:, db * 128 : (db + 1) * 128],
                                attn[j, 2 * qp + 1][:],
                                start=(j == 0),
                                stop=(j == 1),
                            )
                        ot = opool.tile([128, 1024], F16, tag="outev")
                        if db % 2 == 0:
                            nc.vector.tensor_copy(ot[:, 0:512], pA[:])
                            nc.scalar.copy(ot[:, 512:1024], pB[:])
                        else:
                            nc.scalar.copy(ot[:, 0:512], pA[:])
                            nc.vector.tensor_copy(ot[:, 512:1024], pB[:])
                        deng = nc.sync if db % 2 == 0 else nc.gpsimd
                        deng.dma_start(
                            out[db * 128 : (db + 1) * 128, qp * 1024 : (qp + 1) * 1024],
                            ot[:],
                        )

    nc.compile()
    return nc


def _get_nc():
    global _NC
    if _NC is None:
        _NC = _build()
    return _NC


def _in_maps(q, k, v, Wq, bq, Wk, bk, Wv, bv, Wo, bo):
    import ml_dtypes
    f32 = np.float32
    bf16 = ml_dtypes.bfloat16
    maps = []
    for c in range(N_CORES):
        b, hg = c // HG, c % HG
        hs = slice(hg * HG, (hg + 1) * HG)  # this core's 4 heads

        wq_h = np.zeros((D + 1, HG * E), f32)
        wq_h[:D] = np.transpose(Wq[hs], (1, 0, 2)).reshape(D, HG * E)
        wq_h[D] = bq[hs].reshape(-1)
        wk_h = np.zeros((D + 1, HG * E), f32)
        wk_h[:D] = np.transpose(Wk[hs], (1, 0, 2)).reshape(D, HG * E)
        wk_h[D] = bk[hs].reshape(-1)
        wv_h = np.zeros((D + 1, HG * EL), f32)
        for hl in range(HG):
            wv_h[:D, hl * EL : hl * EL + E] = Wv[hg * HG + hl]
            wv_h[D, hl * EL : hl * EL + E] = bv[hg * HG + hl]
            wv_h[D, hl * EL + E] = 1.0  # generates the ones column of vh'
        maps.append(
            {
                "xq": np.ascontiguousarray(q[b].T).astype(bf16),
                "xk": np.ascontiguousarray(k[b].T).astype(bf16),
                "xv": np.ascontiguousarray(v[b].T).astype(bf16),
                "wq": wq_h.astype(bf16),
                "wk": wk_h.astype(bf16),
                "wv": wv_h.astype(bf16),
                "wo": np.ascontiguousarray(
                    Wo[hg * HG * E : (hg + 1) * HG * E, :]
                ).astype(bf16),
                "eall": _EALL,
                "ones": _ONES.astype(bf16),
            }
        )
    return maps


def _run(inputs, trace=False):
    from concourse.bass_utils import run_bass_kernel_spmd

    nc = _get_nc()
    maps = _in_maps(**inputs)
    res = run_bass_kernel_spmd(nc, maps, list(range(N_CORES)), trace=trace)
    bo = np.asarray(inputs["bo"], np.float32)
    out = np.zeros((B, S, D), np.float32)
    for b in range(B):
        acc = np.zeros((D, S), np.float32)
        for hg in range(HG):
            acc += res.results[b * HG + hg]["out_partial"].astype(np.float32)
        out[b] = acc.T + bo[None, :]
    return out, res.exec_time_ns


def kernel(**inputs):
    out, _ = _run(inputs, trace=False)
    return out


def kernel_traced(**inputs):
    return _run(inputs, trace=True)
